# revision 1
# baseline (speedup 1.0000x reference)
"""Trainium2 Bass kernel for the 4-layer tiny CNN (conv5x5+BN+ReLU+AvgPool+Hardtanh x3, conv4x4+BN1d).

Strategy: pure data parallel over batch (1024 -> 128 images per core on 8 cores).
Per core, images are processed in 32 groups of 4. Convs run as float32r matmuls
on the tensor engine with block-diagonal weights packing 4 images into the
128-partition contraction dim; the 25 conv taps accumulate into PSUM via
free-dim offsets on zero-padded SBUF layouts. BN is folded into conv weights
host-side; ReLU+bias+pool-prescale fuse into one scalar-engine activation;
2x2 avg-pool is two strided vector adds; hardtanh clip is a tensor_scalar_min.
"""
import sys
sys.path.insert(0, '/opt/trn_rl_repo')
import numpy as np

EPS = 1e-5
NCORES = 8
BPC = 128          # images per core
NG = 32            # groups of 4 images per core
F1 = 1296          # L1 padded free size (36-wide rows)
OFF1 = 72          # L1 front margin
F2 = 400           # L2 padded image 20x20
F3 = 576           # L3 superblock tile: 12 rows * (4 groups * 12 cols)
NSB = 8            # superblocks (of 4 groups) per core

_NC = None


def _build():
    import concourse.bass as bass
    import concourse.mybir as mybir
    import concourse.tile as tile
    from concourse import bacc

    f32 = mybir.dt.float32
    f32r = mybir.dt.float32r
    Relu = mybir.ActivationFunctionType.Relu
    Identity = mybir.ActivationFunctionType.Identity

    nc = bacc.Bacc("TRN2", target_bir_lowering=False, debug=False)
    x60d = nc.declare_dram_parameter("x60", [NG, 60, F1], f32, isOutput=False)
    w1d = nc.declare_dram_parameter("w1bd", [60, 5 * 128], f32, isOutput=False)
    w2d = nc.declare_dram_parameter("w2bd", [128, 25 * 128], f32, isOutput=False)
    w3d = nc.declare_dram_parameter("w3bd", [128, 50 * 128], f32, isOutput=False)
    w4d = nc.declare_dram_parameter("w4bd", [128, 32 * 40], f32, isOutput=False)
    b1d = nc.declare_dram_parameter("b1r", [128, 1], f32, isOutput=False)
    b2d = nc.declare_dram_parameter("b2r", [128, 1], f32, isOutput=False)
    b3d = nc.declare_dram_parameter("b3r", [128, 2], f32, isOutput=False)
    b4d = nc.declare_dram_parameter("b4r", [40, 1], f32, isOutput=False)
    outd = nc.declare_dram_parameter("out", [40, 32], f32, isOutput=True)

    with tile.TileContext(nc) as tc:
        with tc.tile_pool(name="consts", bufs=1) as cpool, \
             tc.tile_pool(name="persist", bufs=1) as qpool, \
             tc.tile_pool(name="work", bufs=2) as wpool, \
             tc.tile_pool(name="xin", bufs=3) as xpool, \
             tc.tile_pool(name="ps1", bufs=4, space="PSUM") as ps1p, \
             tc.tile_pool(name="ps2", bufs=1, space="PSUM") as ps2p, \
             tc.tile_pool(name="ps3", bufs=2, space="PSUM") as ps3p, \
             tc.tile_pool(name="ps4", bufs=1, space="PSUM") as ps4p:

            # ---- constants ----
            w1sb = cpool.tile([60, 5 * 128], f32r, name="w1sb")
            w2sb = cpool.tile([128, 25 * 128], f32r, name="w2sb")
            w3sb = cpool.tile([128, 50 * 128], f32r, name="w3sb")
            w4sb = cpool.tile([128, 32 * 40], f32, name="w4sb")
            b1sb = cpool.tile([128, 1], f32, name="b1sb")
            b2sb = cpool.tile([128, 1], f32, name="b2sb")
            b3sb = cpool.tile([128, 2], f32, name="b3sb")
            b4sb = cpool.tile([40, 1], f32, name="b4sb")
            nc.sync.dma_start(out=w1sb[:], in_=w1d.ap().bitcast(f32r))
            nc.sync.dma_start(out=w2sb[:], in_=w2d.ap().bitcast(f32r))
            nc.sync.dma_start(out=w3sb[:], in_=w3d.ap().bitcast(f32r))
            nc.sync.dma_start(out=w4sb[:], in_=w4d.ap())
            nc.sync.dma_start(out=b1sb[:], in_=b1d.ap())
            nc.sync.dma_start(out=b2sb[:], in_=b2d.ap())
            nc.sync.dma_start(out=b3sb[:], in_=b3d.ap())
            nc.sync.dma_start(out=b4sb[:], in_=b4d.ap())

            # ---- persistent activation tiles ----
            x2s = [qpool.tile([128, F2], f32r, name=f"x2_{k}") for k in range(3)]
            x3s = [qpool.tile([128, F3], f32r, name=f"x3_{k}") for k in range(2)]
            x4s = [qpool.tile([128, 512], f32, name=f"x4_{h}") for h in range(2)]
            for t in x2s + x3s:
                nc.gpsimd.memset(t[:].bitcast(f32), 0.0)

            AP = bass.AP

            for g in range(NG):
                sb, g4 = divmod(g, 4)
                # ================= L1: conv1 on group g =================
                x60 = xpool.tile([60, F1], f32r, tag="x60", name="x60")
                nc.sync.dma_start(out=x60[:], in_=x60d.ap()[g].bitcast(f32r))

                ps1a = ps1p.tile([128, 512], mybir.dt.float32, tag="ps1", name="ps1a")
                ps1b = ps1p.tile([128, 512], mybir.dt.float32, tag="ps1", name="ps1b")
                for hy, ps in ((0, ps1a), (1, ps1b)):
                    for dx in range(5):
                        rhs = AP(x60.tensor, OFF1 + dx + hy * 16 * 36,
                                 [[F1, 60], [36, 16], [1, 32]])
                        nc.tensor.matmul(ps[:], w1sb[:, dx * 128:(dx + 1) * 128], rhs,
                                         start=(dx == 0), stop=(dx == 4))

                r1 = wpool.tile([128, 1024], f32, tag="r1", name="r1")
                nc.scalar.activation(r1[:, 0:512], ps1a[:], Relu, bias=b1sb[:, 0:1], scale=0.25)
                nc.scalar.activation(r1[:, 512:1024], ps1b[:], Relu, bias=b1sb[:, 0:1], scale=0.25)

                # pool 32x32 -> 16x16, then min(.,1) into padded X2 (20x20, pad 2)
                t1 = wpool.tile([128, 512], f32, tag="t1", name="t1")
                r1v = r1.rearrange("p (a b) -> p a b", a=32)
                nc.vector.tensor_add(t1.rearrange("p (a b) -> p a b", a=32),
                                     r1v[:, :, 0::2], r1v[:, :, 1::2])
                t2 = wpool.tile([128, 256], f32, tag="t2", name="t2")
                t1v = t1.rearrange("p (a b) -> p a b", a=32)
                nc.vector.tensor_add(t2.rearrange("p (a b) -> p a b", a=16),
                                     t1v[:, 0::2, :], t1v[:, 1::2, :])
                x2 = x2s[g % 3]
                nc.vector.tensor_scalar_min(AP(x2.tensor, 42, [[F2, 128], [20, 16], [1, 16]]),
                                            t2.rearrange("p (a b) -> p a b", a=16), 1.0)

                # ================= L2: conv2 on group g =================
                ps2 = ps2p.tile([128, 256], mybir.dt.float32, tag="ps2", name="ps2")
                t = 0
                for dy in range(5):
                    for dx in range(5):
                        rhs = AP(x2.tensor, dy * 20 + dx, [[F2, 128], [20, 16], [1, 16]])
                        nc.tensor.matmul(ps2[:], w2sb[:, t * 128:(t + 1) * 128], rhs,
                                         start=(t == 0), stop=(t == 24))
                        t += 1

                r2 = wpool.tile([128, 256], f32, tag="r2", name="r2")
                nc.scalar.activation(r2[:], ps2[:], Relu, bias=b2sb[:, 0:1], scale=0.25)

                # pool 16x16 -> 8x8, min, into X3 layout [r(12), g4(4), c(12)]
                t3 = wpool.tile([128, 128], f32, tag="t3", name="t3")
                r2v = r2.rearrange("p (a b) -> p a b", a=16)
                nc.vector.tensor_add(t3.rearrange("p (a b) -> p a b", a=16),
                                     r2v[:, :, 0::2], r2v[:, :, 1::2])
                t4 = wpool.tile([128, 64], f32, tag="t4", name="t4")
                t3v = t3.rearrange("p (a b) -> p a b", a=16)
                nc.vector.tensor_add(t4.rearrange("p (a b) -> p a b", a=8),
                                     t3v[:, 0::2, :], t3v[:, 1::2, :])
                x3 = x3s[sb % 2]
                nc.vector.tensor_scalar_min(
                    AP(x3.tensor, 2 * 48 + g4 * 12 + 2, [[F3, 128], [48, 8], [1, 8]]),
                    t4.rearrange("p (a b) -> p a b", a=8), 1.0)

                # ================= L3: conv3 once per superblock =================
                if g4 == 3:
                    for h in range(2):
                        ps3 = ps3p.tile([128, 256], mybir.dt.float32, tag="ps3", name="ps3")
                        t = 0
                        for dy in range(5):
                            for dx in range(5):
                                rhs = AP(x3.tensor, dy * 48 + dx, [[F3, 128], [12, 32], [1, 8]])
                                nc.tensor.matmul(
                                    ps3[:], w3sb[:, (h * 25 + t) * 128:(h * 25 + t + 1) * 128],
                                    rhs, start=(t == 0), stop=(t == 24))
                                t += 1
                        r3 = wpool.tile([128, 256], f32, tag="r3", name="r3")
                        nc.scalar.activation(r3[:], ps3[:], Relu, bias=b3sb[:, h:h + 1], scale=0.25)
                        # pool (oy,g,ox) 8x4x8 -> 4x4x4, min into X4h[(sb*4+g)*16 + py*4 + px]
                        t5 = wpool.tile([128, 128], f32, tag="t5", name="t5")
                        r3v = r3.rearrange("p (a b) -> p a b", a=32)  # [(oy,g)=32, ox=8]
                        nc.vector.tensor_add(t5.rearrange("p (a b) -> p a b", a=32),
                                             r3v[:, :, 0::2], r3v[:, :, 1::2])
                        t6 = wpool.tile([128, 64], f32, tag="t6", name="t6")
                        t5v = t5.rearrange("p (a b c) -> p a b c", a=8, b=4)  # [oy, g, ox2]
                        nc.vector.tensor_add(t6.rearrange("p (a b c) -> p a b c", a=4, b=4),
                                             t5v[:, 0::2, :, :], t5v[:, 1::2, :, :])
                        # t6 logical [py(4), g(4), px(4)] -> X4h offset g*16 + py*4 + px
                        nc.vector.tensor_scalar_min(
                            AP(x4s[h].tensor, sb * 64, [[512, 128], [4, 4], [16, 4], [1, 4]]),
                            t6.rearrange("p (a b c) -> p a b c", a=4, b=4), 1.0)

            # ================= L4: fc conv 4x4 + BN1d, all 128 images =================
            ps4 = ps4p.tile([40, 32], mybir.dt.float32, name="ps4")
            k = 0
            for h in range(2):
                for ty in range(4):
                    for tx in range(4):
                        rhs = AP(x4s[h].tensor, ty * 4 + tx, [[512, 128], [16, 32]])
                        nc.tensor.matmul(ps4[:], w4sb[:, k * 40:(k + 1) * 40], rhs,
                                         start=(k == 0), stop=(k == 31))
                        k += 1
            osb = qpool.tile([40, 32], f32, name="osb")
            nc.scalar.activation(osb[:], ps4[:], Identity, bias=b4sb[:, 0:1], scale=1.0)
            nc.sync.dma_start(out=outd.ap(), in_=osb[:])

    nc.compile()
    return nc


def _fold(g, b, m, v):
    s = (g / np.sqrt(v + EPS)).astype(np.float32)
    return s, (b - m * s).astype(np.float32)


def _prep_consts(w1, g1, b1, m1, v1, w2, g2, b2, m2, v2,
                 w3, g3, b3, m3, v3, w4, g4, b4, m4, v4):
    s1, t1 = _fold(g1, b1, m1, v1)
    s2, t2 = _fold(g2, b2, m2, v2)
    s3, t3 = _fold(g3, b3, m3, v3)
    s4, t4 = _fold(g4, b4, m4, v4)
    w1f = (w1 * s1[:, None, None, None]).astype(np.float32)  # [32,3,5,5]
    w2f = (w2 * s2[:, None, None, None]).astype(np.float32)  # [32,32,5,5]
    w3f = (w3 * s3[:, None, None, None]).astype(np.float32)  # [64,32,5,5]
    w4f = (w4 * s4[:, None, None, None]).astype(np.float32)  # [10,64,4,4]

    # L1: [ (dy,i,ci)=60, dx*128 + (i2*32+co) ]
    w1bd = np.zeros((5, 4, 3, 5, 4, 32), np.float32)  # dy,i,ci,dx,i2,co
    for i in range(4):
        w1bd[:, i, :, :, i, :] = w1f.transpose(2, 1, 3, 0)  # dy,ci,dx,co
    w1bd = w1bd.reshape(60, 5 * 128)

    # L2: [ (i,ci)=128, (dy*5+dx)*128 + (i2*32+co) ]
    w2bd = np.zeros((4, 32, 25, 4, 32), np.float32)  # i,ci,t,i2,co
    wt2 = w2f.transpose(1, 2, 3, 0).reshape(32, 25, 32)  # ci,t,co
    for i in range(4):
        w2bd[i, :, :, i, :] = wt2
    w2bd = w2bd.reshape(128, 25 * 128)

    # L3: [ (i,ci)=128, (h*25+t)*128 + (i2*32+coh) ]
    w3bd = np.zeros((4, 32, 2, 25, 4, 32), np.float32)  # i,ci,h,t,i2,coh
    wt3 = w3f.transpose(1, 2, 3, 0).reshape(32, 25, 2, 32)  # ci,t,h,coh
    for i in range(4):
        w3bd[i, :, :, :, i, :] = wt3.transpose(0, 2, 1, 3)  # ci,h,t,coh
    w3bd = w3bd.reshape(128, 50 * 128)

    # L4: [ (i,c)=128, (h*16 + ty*4+tx)*40 + (i2*10+co) ]
    w4bd = np.zeros((4, 32, 2, 16, 4, 10), np.float32)  # i,c,h,t,i2,co
    wt4 = w4f.reshape(10, 2, 32, 16)  # co,h,c,t
    for i in range(4):
        w4bd[i, :, :, :, i, :] = wt4.transpose(2, 1, 3, 0)  # c,h,t,co
    w4bd = w4bd.reshape(128, 32 * 40)

    b1r = (0.25 * np.tile(t1, 4)).reshape(128, 1).astype(np.float32)
    b2r = (0.25 * np.tile(t2, 4)).reshape(128, 1).astype(np.float32)
    b3r = (0.25 * np.tile(t3.reshape(2, 32), (1, 4)).reshape(2, 128).T).astype(np.float32).copy()
    b4r = np.tile(t4, 4).reshape(40, 1).astype(np.float32)
    return dict(w1bd=w1bd, w2bd=w2bd, w3bd=w3bd, w4bd=w4bd,
                b1r=b1r, b2r=b2r, b3r=np.ascontiguousarray(b3r), b4r=b4r)


def _prep_x60(xc):
    # xc: [128, 3, 32, 32] -> [32, 60, F1]; partition (dy,i,ci), 36-wide padded rows
    xp = np.zeros((NG, 4, 3, 40, 36), np.float32)
    xp[:, :, :, 2:34, 2:34] = xc.reshape(NG, 4, 3, 32, 32)
    xf = xp.reshape(NG, 4, 3, 40 * 36)
    out = np.zeros((NG, 5, 4, 3, F1), np.float32)
    for dy in range(5):
        out[:, dy, :, :, OFF1:OFF1 + 1224] = xf[:, :, :, 36 * dy:36 * dy + 1224]
    return out.reshape(NG, 60, F1)


def kernel(**inputs):
    global _NC
    from concourse.bass_utils import run_bass_kernel_spmd

    x = np.ascontiguousarray(np.asarray(inputs["x"], dtype=np.float32))
    consts = _prep_consts(
        inputs["w1"], inputs["g1"], inputs["b1"], inputs["m1"], inputs["v1"],
        inputs["w2"], inputs["g2"], inputs["b2"], inputs["m2"], inputs["v2"],
        inputs["w3"], inputs["g3"], inputs["b3"], inputs["m3"], inputs["v3"],
        inputs["w4"], inputs["g4"], inputs["b4"], inputs["m4"], inputs["v4"])
    consts = {k: np.ascontiguousarray(v) for k, v in consts.items()}

    if _NC is None:
        _NC = _build()

    in_maps = []
    for c in range(NCORES):
        m = dict(consts)
        m["x60"] = _prep_x60(x[c * BPC:(c + 1) * BPC])
        in_maps.append(m)

    res = run_bass_kernel_spmd(_NC, in_maps, list(range(NCORES)))
    outs = []
    for c in range(NCORES):
        o = res.results[c]["out"].reshape(4, 10, 32)  # [i, co, g]
        outs.append(o.transpose(2, 0, 1).reshape(BPC, 10))  # b = 4g+i
    return np.concatenate(outs, axis=0).astype(np.float32)


# revision 2
# speedup vs baseline: 1.0129x; 1.0129x over previous
"""Trainium2 Bass kernel for the 4-layer tiny CNN (conv5x5+BN+ReLU+AvgPool+Hardtanh x3, conv4x4+BN1d).

Strategy: pure data parallel over batch (1024 -> 128 images per core on 8 cores).
Per core, images are processed in 32 groups of 4. Convs run as float32r matmuls
on the tensor engine with block-diagonal weights packing 4 images into the
128-partition contraction dim; the 25 conv taps accumulate into PSUM via
free-dim offsets on zero-padded SBUF layouts. All matmuls use N=512 so the
weight-load pipeline stays hidden under streaming. BN is folded into conv
weights host-side; ReLU+bias+pool-prescale fuse into one scalar-engine
activation; 2x2 avg-pool is two strided vector adds; hardtanh clip is a
tensor_scalar_min.

Free-dim packing: L1 one group (4 images, 1024 px -> 2 matmul halves of 512);
L2 a pair of groups (8 images) with X2 layout [row20, grp2, col20];
L3 eight groups (32 images) with X3 layout [row12, grp8, col12].
"""
import sys
sys.path.insert(0, '/opt/trn_rl_repo')
import numpy as np

EPS = 1e-5
NCORES = 8
BPC = 128          # images per core
NG = 32            # groups of 4 images per core
NPAIR = 16         # pairs of groups (8 images)
NSB = 4            # superblocks of 8 groups (32 images)
F1 = 1296          # L1 padded free size (36-wide rows)
OFF1 = 72          # L1 front margin
F2 = 800           # L2 pair tile: 20 rows * (2 groups * 20 cols)
F3 = 1152          # L3 superblock tile: 12 rows * (8 groups * 12 cols)

_NC = None


def _build():
    import concourse.bass as bass
    import concourse.mybir as mybir
    import concourse.tile as tile
    from concourse import bacc

    f32 = mybir.dt.float32
    f32r = mybir.dt.float32r
    Relu = mybir.ActivationFunctionType.Relu
    Identity = mybir.ActivationFunctionType.Identity
    AP = bass.AP

    nc = bacc.Bacc("TRN2", target_bir_lowering=False, debug=False)
    x60d = nc.declare_dram_parameter("x60", [NG, 60, F1], f32, isOutput=False)
    w1d = nc.declare_dram_parameter("w1bd", [60, 5 * 128], f32, isOutput=False)
    w2d = nc.declare_dram_parameter("w2bd", [128, 25 * 128], f32, isOutput=False)
    w3d = nc.declare_dram_parameter("w3bd", [128, 50 * 128], f32, isOutput=False)
    w4d = nc.declare_dram_parameter("w4bd", [128, 32 * 40], f32, isOutput=False)
    b1d = nc.declare_dram_parameter("b1r", [128, 1], f32, isOutput=False)
    b2d = nc.declare_dram_parameter("b2r", [128, 1], f32, isOutput=False)
    b3d = nc.declare_dram_parameter("b3r", [128, 2], f32, isOutput=False)
    b4d = nc.declare_dram_parameter("b4r", [40, 1], f32, isOutput=False)
    outd = nc.declare_dram_parameter("out", [40, 32], f32, isOutput=True)

    with tile.TileContext(nc) as tc:
        with tc.tile_pool(name="consts", bufs=1) as cpool, \
             tc.tile_pool(name="persist", bufs=1) as qpool, \
             tc.tile_pool(name="work", bufs=2) as wpool, \
             tc.tile_pool(name="xin", bufs=3) as xpool, \
             tc.tile_pool(name="ps1", bufs=3, space="PSUM") as ps1p, \
             tc.tile_pool(name="ps2", bufs=2, space="PSUM") as ps2p, \
             tc.tile_pool(name="ps3", bufs=2, space="PSUM") as ps3p, \
             tc.tile_pool(name="ps4", bufs=1, space="PSUM") as ps4p:

            # ---- constants ----
            w1sb = cpool.tile([60, 5 * 128], f32r, name="w1sb")
            w2sb = cpool.tile([128, 25 * 128], f32r, name="w2sb")
            w3sb = cpool.tile([128, 50 * 128], f32r, name="w3sb")
            w4sb = cpool.tile([128, 32 * 40], f32, name="w4sb")
            b1sb = cpool.tile([128, 1], f32, name="b1sb")
            b2sb = cpool.tile([128, 1], f32, name="b2sb")
            b3sb = cpool.tile([128, 2], f32, name="b3sb")
            b4sb = cpool.tile([40, 1], f32, name="b4sb")
            nc.sync.dma_start(out=w1sb[:], in_=w1d.ap().bitcast(f32r))
            nc.sync.dma_start(out=w2sb[:], in_=w2d.ap().bitcast(f32r))
            nc.sync.dma_start(out=w3sb[:], in_=w3d.ap().bitcast(f32r))
            nc.sync.dma_start(out=w4sb[:], in_=w4d.ap())
            nc.sync.dma_start(out=b1sb[:], in_=b1d.ap())
            nc.sync.dma_start(out=b2sb[:], in_=b2d.ap())
            nc.sync.dma_start(out=b3sb[:], in_=b3d.ap())
            nc.sync.dma_start(out=b4sb[:], in_=b4d.ap())

            # ---- persistent activation tiles ----
            x2s = [qpool.tile([128, F2], f32r, name=f"x2_{k}") for k in range(3)]
            x3s = [qpool.tile([128, F3], f32r, name=f"x3_{k}") for k in range(2)]
            x4s = [qpool.tile([128, 512], f32, name=f"x4_{h}") for h in range(2)]
            for t in x2s + x3s:
                nc.gpsimd.memset(t[:].bitcast(f32), 0.0)

            for g in range(NG):
                j, gp = divmod(g, 2)      # pair index, position in pair
                sb, g8 = divmod(g, 8)     # superblock index, position in sb
                # ================= L1: conv1 on group g (4 images) =================
                x60 = xpool.tile([60, F1], f32r, tag="x60", name="x60")
                nc.sync.dma_start(out=x60[:], in_=x60d.ap()[g].bitcast(f32r))

                ps1a = ps1p.tile([128, 512], mybir.dt.float32, tag="ps1", name="ps1a")
                ps1b = ps1p.tile([128, 512], mybir.dt.float32, tag="ps1", name="ps1b")
                for hy, ps in ((0, ps1a), (1, ps1b)):
                    for dx in range(5):
                        rhs = AP(x60.tensor, OFF1 + dx + hy * 16 * 36,
                                 [[F1, 60], [36, 16], [1, 32]])
                        nc.tensor.matmul(ps[:], w1sb[:, dx * 128:(dx + 1) * 128], rhs,
                                         start=(dx == 0), stop=(dx == 4))

                r1 = wpool.tile([128, 1024], f32, tag="r1", name="r1")
                nc.scalar.activation(r1[:, 0:512], ps1a[:], Relu, bias=b1sb[:, 0:1], scale=0.25)
                nc.scalar.activation(r1[:, 512:1024], ps1b[:], Relu, bias=b1sb[:, 0:1], scale=0.25)

                # pool 32x32 -> 16x16, min(.,1) into padded X2 pair tile [r20, gp2, c20]
                t1 = wpool.tile([128, 512], f32, tag="t1", name="t1")
                r1v = r1.rearrange("p (a b) -> p a b", a=32)
                nc.vector.tensor_add(t1.rearrange("p (a b) -> p a b", a=32),
                                     r1v[:, :, 0::2], r1v[:, :, 1::2])
                t2 = wpool.tile([128, 256], f32, tag="t2", name="t2")
                t1v = t1.rearrange("p (a b) -> p a b", a=32)
                nc.vector.tensor_add(t2.rearrange("p (a b) -> p a b", a=16),
                                     t1v[:, 0::2, :], t1v[:, 1::2, :])
                x2 = x2s[j % 3]
                nc.vector.tensor_scalar_min(
                    AP(x2.tensor, 2 * 40 + gp * 20 + 2, [[F2, 128], [40, 16], [1, 16]]),
                    t2.rearrange("p (a b) -> p a b", a=16), 1.0)

                if gp != 1:
                    continue
                # ================= L2: conv2 on pair j (8 images) =================
                ps2 = ps2p.tile([128, 512], mybir.dt.float32, tag="ps2", name="ps2")
                t = 0
                for dy in range(5):
                    for dx in range(5):
                        rhs = AP(x2.tensor, dy * 40 + dx, [[F2, 128], [20, 32], [1, 16]])
                        nc.tensor.matmul(ps2[:], w2sb[:, t * 128:(t + 1) * 128], rhs,
                                         start=(t == 0), stop=(t == 24))
                        t += 1

                r2 = wpool.tile([128, 512], f32, tag="r2", name="r2")
                nc.scalar.activation(r2[:], ps2[:], Relu, bias=b2sb[:, 0:1], scale=0.25)

                # pool (oy16, gp2, ox16) -> (8, 2, 8), min into X3 [r12, g8, c12]
                t3 = wpool.tile([128, 256], f32, tag="t3", name="t3")
                r2v = r2.rearrange("p (a b) -> p a b", a=32)   # [(oy,gp)=32, ox=16]
                nc.vector.tensor_add(t3.rearrange("p (a b) -> p a b", a=32),
                                     r2v[:, :, 0::2], r2v[:, :, 1::2])
                t4 = wpool.tile([128, 128], f32, tag="t4", name="t4")
                t3v = t3.rearrange("p (a b c) -> p a b c", a=16, b=2)  # [oy, gp, ox2]
                nc.vector.tensor_add(t4.rearrange("p (a b c) -> p a b c", a=8, b=2),
                                     t3v[:, 0::2, :, :], t3v[:, 1::2, :, :])
                x3 = x3s[sb % 2]
                nc.vector.tensor_scalar_min(
                    AP(x3.tensor, 2 * 96 + ((2 * j) % 8) * 12 + 2,
                       [[F3, 128], [96, 8], [12, 2], [1, 8]]),
                    t4.rearrange("p (a b c) -> p a b c", a=8, b=2), 1.0)

                if g8 != 7:
                    continue
                # ================= L3: conv3 on superblock sb (32 images) =================
                for h in range(2):
                    ps3 = ps3p.tile([128, 512], mybir.dt.float32, tag="ps3", name="ps3")
                    t = 0
                    for dy in range(5):
                        for dx in range(5):
                            rhs = AP(x3.tensor, dy * 96 + dx, [[F3, 128], [12, 64], [1, 8]])
                            nc.tensor.matmul(
                                ps3[:], w3sb[:, (h * 25 + t) * 128:(h * 25 + t + 1) * 128],
                                rhs, start=(t == 0), stop=(t == 24))
                            t += 1
                    r3 = wpool.tile([128, 512], f32, tag="r3", name="r3")
                    nc.scalar.activation(r3[:], ps3[:], Relu, bias=b3sb[:, h:h + 1], scale=0.25)
                    # pool (oy8, g8, ox8) -> (4, 8, 4), min into X4h[G*16 + py*4 + px]
                    t5 = wpool.tile([128, 256], f32, tag="t5", name="t5")
                    r3v = r3.rearrange("p (a b) -> p a b", a=64)   # [(oy,g)=64, ox=8]
                    nc.vector.tensor_add(t5.rearrange("p (a b) -> p a b", a=64),
                                         r3v[:, :, 0::2], r3v[:, :, 1::2])
                    t6 = wpool.tile([128, 128], f32, tag="t6", name="t6")
                    t5v = t5.rearrange("p (a b c) -> p a b c", a=8, b=8)  # [oy, g, ox2]
                    nc.vector.tensor_add(t6.rearrange("p (a b c) -> p a b c", a=4, b=8),
                                         t5v[:, 0::2, :, :], t5v[:, 1::2, :, :])
                    # t6 logical [py(4), g(8), px(4)] -> X4h offset G*16 + py*4 + px
                    nc.vector.tensor_scalar_min(
                        AP(x4s[h].tensor, sb * 128, [[512, 128], [4, 4], [16, 8], [1, 4]]),
                        t6.rearrange("p (a b c) -> p a b c", a=4, b=8), 1.0)

            # ================= L4: fc conv 4x4 + BN1d, all 128 images =================
            ps4 = ps4p.tile([40, 32], mybir.dt.float32, name="ps4")
            k = 0
            for h in range(2):
                for ty in range(4):
                    for tx in range(4):
                        rhs = AP(x4s[h].tensor, ty * 4 + tx, [[512, 128], [16, 32]])
                        nc.tensor.matmul(ps4[:], w4sb[:, k * 40:(k + 1) * 40], rhs,
                                         start=(k == 0), stop=(k == 31))
                        k += 1
            osb = qpool.tile([40, 32], f32, name="osb")
            nc.scalar.activation(osb[:], ps4[:], Identity, bias=b4sb[:, 0:1], scale=1.0)
            nc.sync.dma_start(out=outd.ap(), in_=osb[:])

    nc.compile()
    return nc


def _fold(g, b, m, v):
    s = (g / np.sqrt(v + EPS)).astype(np.float32)
    return s, (b - m * s).astype(np.float32)


def _prep_consts(w1, g1, b1, m1, v1, w2, g2, b2, m2, v2,
                 w3, g3, b3, m3, v3, w4, g4, b4, m4, v4):
    s1, t1 = _fold(g1, b1, m1, v1)
    s2, t2 = _fold(g2, b2, m2, v2)
    s3, t3 = _fold(g3, b3, m3, v3)
    s4, t4 = _fold(g4, b4, m4, v4)
    w1f = (w1 * s1[:, None, None, None]).astype(np.float32)  # [32,3,5,5]
    w2f = (w2 * s2[:, None, None, None]).astype(np.float32)  # [32,32,5,5]
    w3f = (w3 * s3[:, None, None, None]).astype(np.float32)  # [64,32,5,5]
    w4f = (w4 * s4[:, None, None, None]).astype(np.float32)  # [10,64,4,4]

    # L1: [ (dy,i,ci)=60, dx*128 + (i2*32+co) ]
    w1bd = np.zeros((5, 4, 3, 5, 4, 32), np.float32)  # dy,i,ci,dx,i2,co
    for i in range(4):
        w1bd[:, i, :, :, i, :] = w1f.transpose(2, 1, 3, 0)  # dy,ci,dx,co
    w1bd = w1bd.reshape(60, 5 * 128)

    # L2: [ (i,ci)=128, (dy*5+dx)*128 + (i2*32+co) ]
    w2bd = np.zeros((4, 32, 25, 4, 32), np.float32)  # i,ci,t,i2,co
    wt2 = w2f.transpose(1, 2, 3, 0).reshape(32, 25, 32)  # ci,t,co
    for i in range(4):
        w2bd[i, :, :, i, :] = wt2
    w2bd = w2bd.reshape(128, 25 * 128)

    # L3: [ (i,ci)=128, (h*25+t)*128 + (i2*32+coh) ]
    w3bd = np.zeros((4, 32, 2, 25, 4, 32), np.float32)  # i,ci,h,t,i2,coh
    wt3 = w3f.transpose(1, 2, 3, 0).reshape(32, 25, 2, 32)  # ci,t,h,coh
    for i in range(4):
        w3bd[i, :, :, :, i, :] = wt3.transpose(0, 2, 1, 3)  # ci,h,t,coh
    w3bd = w3bd.reshape(128, 50 * 128)

    # L4: [ (i,c)=128, (h*16 + ty*4+tx)*40 + (i2*10+co) ]
    w4bd = np.zeros((4, 32, 2, 16, 4, 10), np.float32)  # i,c,h,t,i2,co
    wt4 = w4f.reshape(10, 2, 32, 16)  # co,h,c,t
    for i in range(4):
        w4bd[i, :, :, :, i, :] = wt4.transpose(2, 1, 3, 0)  # c,h,t,co
    w4bd = w4bd.reshape(128, 32 * 40)

    b1r = (0.25 * np.tile(t1, 4)).reshape(128, 1).astype(np.float32)
    b2r = (0.25 * np.tile(t2, 4)).reshape(128, 1).astype(np.float32)
    b3r = (0.25 * np.tile(t3.reshape(2, 32), (1, 4)).reshape(2, 128).T).astype(np.float32)
    b4r = np.tile(t4, 4).reshape(40, 1).astype(np.float32)
    return dict(w1bd=w1bd, w2bd=w2bd, w3bd=w3bd, w4bd=w4bd,
                b1r=b1r, b2r=b2r, b3r=np.ascontiguousarray(b3r), b4r=b4r)


def _prep_x60(xc):
    # xc: [128, 3, 32, 32] -> [32, 60, F1]; partition (dy,i,ci), 36-wide padded rows
    xp = np.zeros((NG, 4, 3, 40, 36), np.float32)
    xp[:, :, :, 2:34, 2:34] = xc.reshape(NG, 4, 3, 32, 32)
    xf = xp.reshape(NG, 4, 3, 40 * 36)
    out = np.zeros((NG, 5, 4, 3, F1), np.float32)
    for dy in range(5):
        out[:, dy, :, :, OFF1:OFF1 + 1224] = xf[:, :, :, 36 * dy:36 * dy + 1224]
    return out.reshape(NG, 60, F1)


def kernel(**inputs):
    global _NC
    from concourse.bass_utils import run_bass_kernel_spmd

    x = np.ascontiguousarray(np.asarray(inputs["x"], dtype=np.float32))
    consts = _prep_consts(
        inputs["w1"], inputs["g1"], inputs["b1"], inputs["m1"], inputs["v1"],
        inputs["w2"], inputs["g2"], inputs["b2"], inputs["m2"], inputs["v2"],
        inputs["w3"], inputs["g3"], inputs["b3"], inputs["m3"], inputs["v3"],
        inputs["w4"], inputs["g4"], inputs["b4"], inputs["m4"], inputs["v4"])
    consts = {k: np.ascontiguousarray(v) for k, v in consts.items()}

    if _NC is None:
        _NC = _build()

    in_maps = []
    for c in range(NCORES):
        m = dict(consts)
        m["x60"] = _prep_x60(x[c * BPC:(c + 1) * BPC])
        in_maps.append(m)

    res = run_bass_kernel_spmd(_NC, in_maps, list(range(NCORES)))
    outs = []
    for c in range(NCORES):
        o = res.results[c]["out"].reshape(4, 10, 32)  # [i, co, g]
        outs.append(o.transpose(2, 0, 1).reshape(BPC, 10))  # b = 4g+i
    return np.concatenate(outs, axis=0).astype(np.float32)
